# revision 7
# baseline (speedup 1.0000x reference)
"""Trainium2 Bass kernel for nn_Encoder (GNN message passing, 2 graphs).

Strategy (8-core SPMD, sharded dense + AllGather):
  - Nodes sharded into 8 contiguous ranges of 6272 (= 50176/8, padded);
    core c owns nodes [6272c, 6272(c+1)) for the dense embed/qkv phase,
    the edges whose src falls in that range, and the output rows.
  - Dense phase is SHARDED: each core embeds only its 6272-row x slice
    (Linear-ReLU-Linear-ReLU + qkv projection) and writes a local
    [6272, 768] (q|k|v per graph) table, then an HBM->HBM AllGather
    builds the full [50176, 768] table on every core.
  - Sparse phase: per (graph, dst-half bucket): dma_gather q rows from
    the LOCAL table (int16 local indices), k|v rows from the gathered
    table (two half-table bases so indices fit int16).  Scores via
    tensor ops + exp on ACT, then a w-scaled selector matrix
    S'[e,n] = w_e * (src_rel_e == n) built with one tensor_scalar op per
    128-edge tile.  Aggregation uT[f,n] += V_tile^T-as-lhsT @ S' on the
    PE into PSUM per 128-node group; denominators via ones-lhsT @ S'.
  - Gather index tables are shipped in the compact 16-partition wrap
    layout and replicated to 128 partitions on-device (8 stripe DMAs),
    srel ships as bf16 and is upcast on device; y_out returns bf16.
  - Phase D (output MLP) on the core's own 6272 rows; host concatenates.
"""

import math
import numpy as np
import ml_dtypes

BF = ml_dtypes.bfloat16

N = 50000
NG = 2
NE = 800000
C = 8
NSH = 6272            # nodes per core = 49 * 128
NPAD = NSH * C        # 50176
NGR = 49              # 128-node groups per core
GRP = 128
HALF = NPAD // 2      # 25088 (= 196*128), dst bucket size
SC = 4096             # edges per sparse chunk
TPC = 32              # 128-edge tiles per chunk
SCALE = float(1.0 / math.sqrt(128.0))
PAD_SREL = 255.0      # outside [0,128) -> selector row is all zeros


def _pack_edges(edge_index):
    """Host-side packing. Returns (TPG, NCk, qidx, kvidx, srel) where
    qidx/kvidx are int16 [C, NG, 2, 16, NCk*256] in compact dma_gather wrap
    layout (idx i of chunk ck at [i%16, ck*256 + i//16]) and srel is bf16
    [C, NG, 2, 128, NCk*TPC] with edge e of chunk ck at [e%128, ck*TPC+e//128]."""
    ei = np.asarray(edge_index).astype(np.int64)
    per = {}
    counts = []
    for g in range(NG):
        src, dst = ei[g, 0], ei[g, 1]
        order = np.argsort(src, kind="stable")
        src, dst = src[order], dst[order]
        core_of = src // NSH
        core_starts = np.searchsorted(core_of, np.arange(C + 1))
        for c in range(C):
            s = slice(core_starts[c], core_starts[c + 1])
            s_loc = src[s] - c * NSH
            d = dst[s]
            for b in range(2):
                bsel = (d // HALF) == b
                sl = s_loc[bsel]
                dl = d[bsel] - b * HALF
                grp = sl // GRP
                cnt = np.bincount(grp, minlength=NGR)
                counts.append(cnt)
                per[(g, c, b)] = (sl, dl, grp, cnt)
    TPG = int(max(int(cnt.max()) for cnt in counts) + 127) // 128
    NTILES = NGR * TPG
    NCk = (NTILES + TPC - 1) // TPC
    CAP = NCk * TPC * 128

    qidx = np.zeros((C, NG, 2, CAP), np.int16)
    kvidx = np.zeros((C, NG, 2, CAP), np.int16)
    srel = np.full((C, NG, 2, CAP), PAD_SREL, np.float32)
    for (g, c, b), (sl, dl, grp, cnt) in per.items():
        # slot = grp*TPG*128 + rank within group (edges already sorted by src
        # => sorted by grp; rank = position - group start)
        gstart = np.concatenate([[0], np.cumsum(cnt)[:-1]])
        pos = np.arange(len(sl)) - gstart[grp]
        slot = grp * (TPG * 128) + pos
        qidx[c, g, b, slot] = sl.astype(np.int16)
        kvidx[c, g, b, slot] = dl.astype(np.int16)
        srel[c, g, b, slot] = (sl - grp * GRP).astype(np.float32)

    # compact wrap: [C,NG,2,CAP] -> [C,NG,2,16,NCk*256], idx i of chunk ck at
    # [i%16, ck*256 + i//16]
    def wrap_idx(a):
        a = a.reshape(C, NG, 2, NCk, SC // 16, 16)
        a = np.swapaxes(a, -1, -2)                     # [C,NG,2,NCk,16,256]
        a = np.swapaxes(a, -2, -3)                     # [C,NG,2,16,NCk,256]
        return a.reshape(C, NG, 2, 16, NCk * (SC // 16)).copy()

    def wrap_srel(a):  # -> [C,NG,2,128,NCk*TPC], e of chunk ck at [e%128, ck*TPC+e//128]
        a = a.reshape(C, NG, 2, NCk, TPC, 128)
        a = np.swapaxes(a, -1, -2)                     # [C,NG,2,NCk,128,TPC]
        a = np.swapaxes(a, -2, -3)                     # [C,NG,2,128,NCk,TPC]
        return a.reshape(C, NG, 2, 128, NCk * TPC).astype(np.uint8).copy()

    return TPG, NCk, wrap_idx(qidx), wrap_idx(kvidx), wrap_srel(srel)


def _build_program(TPG, NCk, consts):
    import concourse.bass as bass
    import concourse.bacc as bacc
    import concourse.tile as tile
    import concourse.mybir as mybir
    from concourse.alu_op_type import AluOpType
    from concourse import library_config
    import bass_rust

    AF = bass_rust.ActivationFunctionType
    dt = mybir.dt
    bf16, f32, i16, u8 = dt.bfloat16, dt.float32, dt.int16, dt.uint8

    nc = bacc.Bacc("TRN2", target_bir_lowering=False, debug=False,
                   num_devices=C)

    ICOLS = NCk * (SC // 16)     # idx cols per (g,b)
    WCOLS = NCk * TPC            # srel cols per (g,b)
    NTILES_TOT = NCk * TPC

    # ---- I/O (weights ride in the NEFF as Const tensors) ----
    x_sh = nc.dram_tensor("x_sh", [NSH, 128], bf16, kind="ExternalInput")
    W1 = nc.inline_tensor(consts["W1"], "W1")
    W2 = nc.inline_tensor(consts["W2"], "W2")
    Watt = nc.inline_tensor(consts["Watt"], "Watt")
    b1 = nc.inline_tensor(consts["b1"], "b1")
    b2 = nc.inline_tensor(consts["b2"], "b2")
    battrf = nc.inline_tensor(consts["battrf"], "battrf")
    battrb = nc.inline_tensor(consts["battrb"], "battrb")
    Wo1 = nc.inline_tensor(consts["Wo1"], "Wo1")
    bo1 = nc.inline_tensor(consts["bo1"], "bo1")
    Wo2 = nc.inline_tensor(consts["Wo2"], "Wo2")
    bo2r = nc.inline_tensor(consts["bo2r"], "bo2r")
    iota_t = nc.inline_tensor(consts["iota"], "iota")
    ones_e = nc.inline_tensor(consts["ones_e"], "ones_e")
    ones_r = nc.inline_tensor(consts["ones_r"], "ones_r")
    qidx_t = nc.dram_tensor("qidx", [NG, 2, 16, ICOLS], i16,
                            kind="ExternalInput")
    kvidx_t = nc.dram_tensor("kvidx", [NG, 2, 16, ICOLS], i16,
                             kind="ExternalInput")
    srel_t = nc.dram_tensor("srel", [NG, 2, 128, WCOLS], u8,
                            kind="ExternalInput")
    y_out = nc.dram_tensor("y_out", [NSH, 64], bf16, kind="ExternalOutput")

    bt_loc = nc.dram_tensor("bt_loc", [NSH, 768], bf16, kind="Internal")
    bt_all = nc.dram_tensor("bt_all", [NPAD, 768], bf16, kind="Internal",
                            addr_space="Shared")
    hT_loc = nc.dram_tensor("hT_loc", [128, NSH], bf16, kind="Internal")

    dense_chunks = [(0, 4096), (4096, NSH - 4096)]

    with tile.TileContext(nc) as tc:
        with (
            tc.tile_pool(name="cp", bufs=1) as cp,
            tc.tile_pool(name="dp", bufs=2) as dp,
            tc.tile_pool(name="up", bufs=1) as up,
        ):
            # ---- load consts ----
            def cload(t, shape, dtp):
                s = cp.tile(shape, dtp, tag=t.name, name=t.name + "_s")
                nc.sync.dma_start(s[:], t.ap()[:])
                return s
            W1_s = cload(W1, [128, 128], bf16)
            W2_s = cload(W2, [128, 128], bf16)
            Watt_s = cload(Watt, [128, 768], bf16)
            b1_s = cload(b1, [128, 1], f32)
            b2_s = cload(b2, [128, 1], f32)
            battrf_s = cload(battrf, [128, 384], f32)
            battrb_s = cload(battrb, [128, 384], bf16)
            Wo1_s = []
            for _i in range(3):
                _w = cp.tile([128, 128], bf16, tag=f"Wo1_{_i}", name=f"Wo1_{_i}")
                nc.sync.dma_start(_w[:], Wo1.ap()[128 * _i:128 * (_i + 1), :])
                Wo1_s.append(_w)
            bo1_s = cload(bo1, [128, 1], f32)
            Wo2_s = cload(Wo2, [128, 64], bf16)
            bo2r_s = cload(bo2r, [128, 64], f32)
            iota_s = cload(iota_t, [128, 128], bf16)
            ones_e_s = cload(ones_e, [128, 1], bf16)
            ones_r_s = cload(ones_r, [1, 128], f32)
            nc.gpsimd.load_library(library_config.standard)
            nc.gpsimd.load_library(library_config.standard)

            # ========== PHASE AB (dense, this core's 6272-row slice) ==========
            ab_scope = tc.tile_pool(name="dd", bufs=2)
            dd = ab_scope.__enter__()
            psab_scope = tc.tile_pool(name="psab", bufs=2, space="PSUM")
            ps = psab_scope.__enter__()
            for (r0, nr) in dense_chunks:
                ntile = nr // 128
                xT = dd.tile([128, nr], bf16, tag="xT", name="xT")
                nc.sync.dma_start_transpose(
                    xT[:, 0:nr], x_sh.ap()[r0:r0 + nr, :])
                h1T = dd.tile([128, nr], bf16, tag="h1T", name="h1T")
                for j in range((nr + 511) // 512):
                    wd = min(512, nr - 512 * j)
                    psA = ps.tile([128, 512], f32, tag="psA", name="psA")
                    nc.tensor.matmul(psA[:, :wd], W1_s[:],
                                     xT[:, 512 * j:512 * j + wd],
                                     start=True, stop=True)
                    nc.scalar.activation(h1T[:, 512 * j:512 * j + wd],
                                         psA[:, :wd], AF.Relu, bias=b1_s[:])
                hT = dd.tile([128, nr], bf16, tag="hT", name="hT")
                for j in range((nr + 511) // 512):
                    wd = min(512, nr - 512 * j)
                    psA = ps.tile([128, 512], f32, tag="psA", name="psA")
                    nc.tensor.matmul(psA[:, :wd], W2_s[:],
                                     h1T[:, 512 * j:512 * j + wd],
                                     start=True, stop=True)
                    nc.scalar.activation(hT[:, 512 * j:512 * j + wd],
                                         psA[:, :wd], AF.Relu, bias=b2_s[:])
                nc.sync.dma_start(hT_loc.ap()[:, r0:r0 + nr], hT[:])
                for t in range(ntile):
                    psB = ps.tile([128, 768], f32, tag="psB", name="psB")
                    hTt = hT[:, 128 * t:128 * (t + 1)]
                    nc.tensor.matmul(psB[:, 0:512], hTt, Watt_s[:, 0:512],
                                     start=True, stop=True)
                    nc.tensor.matmul(psB[:, 512:768], hTt, Watt_s[:, 512:768],
                                     start=True, stop=True)
                    ab = dd.tile([128, 768], bf16, tag="ab", name="ab")
                    nc.vector.tensor_tensor(ab[:, 0:384], psB[:, 0:384],
                                            battrf_s[:], AluOpType.add)
                    nc.scalar.activation(ab[:, 384:768], psB[:, 384:768],
                                         AF.Copy)
                    nc.gpsimd.tensor_tensor(ab[:, 384:768], ab[:, 384:768],
                                            battrb_s[:], AluOpType.add)
                    nc.sync.dma_start(
                        bt_loc.ap()[r0 + 128 * t: r0 + 128 * (t + 1), :],
                        ab[:])

            psab_scope.__exit__(None, None, None)
            ab_scope.__exit__(None, None, None)
            tc.strict_bb_all_engine_barrier()

            # ========== ALLGATHER (q|k|v table across the 8 cores) ==========
            nc.gpsimd.collective_compute(
                "AllGather", mybir.AluOpType.bypass,
                replica_groups=[list(range(C))],
                ins=[bt_loc.ap()[:, :]],
                outs=[bt_all.ap()[:, :]],
            )
            tc.strict_bb_all_engine_barrier()
            nc.gpsimd.load_library(library_config.attnmlp)

            # ================= SPARSE PHASE =================
            sp_scope = tc.tile_pool(name="sp", bufs=2)
            sp = sp_scope.__enter__()
            pssp_scope = tc.tile_pool(name="pssp", bufs=2, space="PSUM")
            psu = pssp_scope.__enter__()
            uT = [None, None]
            s_row = [None, None]
            x1T = [None, None]
            for g in range(NG):
                uT[g] = up.tile([128, NSH], f32, tag="uT", name=f"uT{g}")
                s_row[g] = up.tile([1, NSH], f32, tag="s", name=f"s{g}")
                for b in range(2):
                    # full-(g,b) index tables: replicate 16-row wrap to 128
                    qfull = sp.tile([128, ICOLS], i16, tag="qfull",
                                    name="qfull", bufs=1)
                    kfull = sp.tile([128, ICOLS], i16, tag="kfull",
                                    name="kfull", bufs=1)
                    for r in range(8):
                        nc.sync.dma_start(qfull[16 * r:16 * r + 16, :],
                                          qidx_t.ap()[g, b])
                        nc.sync.dma_start(kfull[16 * r:16 * r + 16, :],
                                          kvidx_t.ap()[g, b])
                    srb = sp.tile([128, WCOLS], u8, tag="srb", name="srb",
                                  bufs=1)
                    nc.sync.dma_start(srb[:], srel_t.ap()[g, b])
                    srf = sp.tile([128, WCOLS], f32, tag="srf", name="srf",
                                  bufs=1)
                    nc.vector.tensor_copy(srf[:], srb[:])

                    cur_psU = {}
                    cur_psS = {}
                    for ck in range(NCk):
                        Q = sp.tile([128, TPC, 128], bf16, tag="Q", name="Q",
                                    bufs=3)
                        nc.gpsimd.dma_gather(
                            Q[:], bt_loc.ap()[:, 384 * g:384 * g + 128],
                            qfull[:, 256 * ck:256 * (ck + 1)], SC, SC, 128,
                            elem_step=768, single_packet=False)
                        KV = sp.tile([128, TPC, 256], bf16, tag="KV",
                                     name="KV", bufs=2)
                        nc.gpsimd.dma_gather(
                            KV[:],
                            bt_all.ap()[HALF * b: HALF * (b + 1),
                                        384 * g + 128:384 * g + 384],
                            kfull[:, 256 * ck:256 * (ck + 1)], SC, SC, 256,
                            elem_step=768, single_packet=False)

                        sc_f = sp.tile([128, TPC], f32, tag="scf", name="scf")
                        qk = sp.tile([128, TPC, 128], bf16, tag="qk",
                                     name="qk", bufs=2)
                        nc.vector.tensor_tensor(qk[:], Q[:], KV[:, :, 0:128],
                                                AluOpType.mult)
                        for hw_ in (64, 32, 16):
                            nc.vector.tensor_tensor(
                                qk[:, :, 0:hw_], qk[:, :, 0:hw_],
                                qk[:, :, hw_:2 * hw_], AluOpType.add)
                        nc.vector.tensor_reduce(sc_f[:], qk[:, :, 0:16],
                                                mybir.AxisListType.X,
                                                AluOpType.add)
                        w = sp.tile([128, TPC], f32, tag="w", name="w")
                        nc.scalar.activation(w[:], sc_f[:], AF.Exp,
                                             scale=SCALE)
                        Sp = sp.tile([128, TPC, 128], bf16, tag="Sp",
                                     name="Sp", bufs=1)
                        for t in range(TPC):
                            nc.vector.tensor_scalar(
                                Sp[:, t, :], iota_s[:],
                                srf[:, TPC * ck + t:TPC * ck + t + 1],
                                w[:, t:t + 1], AluOpType.is_equal,
                                AluOpType.mult)
                        for t in range(TPC):
                            tau = ck * TPC + t
                            G = min(tau // TPG, NGR - 1)
                            first = (tau == G * TPG)
                            last = (tau == ((G + 1) * TPG - 1 if G < NGR - 1
                                            else NTILES_TOT - 1))
                            if first:
                                cur_psU[G] = psu.tile([128, 128], f32,
                                                      tag="psU", name="psU")
                                cur_psS[G] = psu.tile([1, 128], f32,
                                                      tag="psS", name="psS")
                            nc.tensor.matmul(cur_psU[G][:], KV[:, t, 128:256],
                                             Sp[:, t, :], start=first,
                                             stop=last)
                            nc.tensor.matmul(cur_psS[G][:], ones_e_s[:],
                                             Sp[:, t, :], start=first,
                                             stop=last)
                            if last:
                                u_dst = uT[g][:, 128 * G:128 * (G + 1)]
                                s_dst = s_row[g][0:1, 128 * G:128 * (G + 1)]
                                if b == 0:
                                    nc.vector.tensor_copy(u_dst, cur_psU[G][:])
                                    nc.scalar.copy(s_dst, cur_psS[G][:])
                                else:
                                    nc.vector.tensor_tensor(
                                        u_dst, cur_psU[G][:], u_dst,
                                        AluOpType.add)
                                    nc.vector.tensor_tensor(
                                        s_dst, cur_psS[G][:], s_dst,
                                        AluOpType.add)
                # normalize graph g -> x1T
                x1T[g] = up.tile([128, NSH], bf16, tag=f"x1T{g}", name=f"x1T{g}")
                for blk in range((NSH + 511) // 512):
                    wd = min(512, NSH - 512 * blk)
                    rcp = dp.tile([1, 512], f32, tag="rcp", name="rcp")
                    nc.vector.reciprocal_approx_fast(
                        rcp[0:1, :wd], s_row[g][0:1, 512 * blk:512 * blk + wd])
                    psR = psu.tile([128, 512], f32, tag="psR", name="psR")
                    nc.tensor.matmul(psR[:, :wd], ones_r_s[:],
                                     rcp[0:1, :wd],
                                     start=True, stop=True)
                    nc.vector.tensor_tensor(
                        x1T[g][:, 512 * blk:512 * blk + wd],
                        uT[g][:, 512 * blk:512 * blk + wd],
                        psR[:, :wd], AluOpType.mult)

            pssp_scope.__exit__(None, None, None)
            sp_scope.__exit__(None, None, None)
            tc.strict_bb_all_engine_barrier()

            # ================= PHASE D =================
            psd_scope = tc.tile_pool(name="psd", bufs=2, space="PSUM")
            psd = psd_scope.__enter__()
            h_sl = up.tile([128, NSH], bf16, tag="h_sl", name="h_sl")
            nc.sync.dma_start(h_sl[:], hT_loc.ap()[:, :])
            for nt in range(NGR):
                sl = slice(128 * nt, 128 * (nt + 1))
                psZ = psd.tile([128, 128], f32, tag="psZ", name="psZ")
                nc.tensor.matmul(psZ[:], Wo1_s[0], h_sl[:, sl],
                                 start=True, stop=False)
                nc.tensor.matmul(psZ[:], Wo1_s[1], x1T[0][:, sl],
                                 start=False, stop=False)
                nc.tensor.matmul(psZ[:], Wo1_s[2], x1T[1][:, sl],
                                 start=False, stop=True)
                zT = dp.tile([128, 128], bf16, tag="zT", name="zT")
                nc.scalar.activation(zT[:], psZ[:], AF.Relu, bias=bo1_s[:])
                psY = psd.tile([128, 64], f32, tag="psY", name="psY")
                nc.tensor.matmul(psY[:], zT[:], Wo2_s[:], start=True,
                                 stop=True)
                ysb = dp.tile([128, 64], bf16, tag="ysb", name="ysb")
                nc.vector.tensor_tensor(ysb[:], psY[:], bo2r_s[:],
                                        AluOpType.add)
                nc.sync.dma_start(y_out.ap()[sl, :], ysb[:])
            psd_scope.__exit__(None, None, None)

    nc.compile()
    return nc


def _make_consts(inputs):
    W_att = np.asarray(inputs["W_att"], np.float32)
    b_att = np.asarray(inputs["b_att"], np.float32)
    battr_rep = np.broadcast_to(b_att[None, :], (128, 768)).copy()
    return {
        "W1": np.asarray(inputs["W_e1"]).astype(BF),
        "W2": np.asarray(inputs["W_e2"]).astype(BF),
        "Watt": W_att.astype(BF),
        "b1": np.asarray(inputs["b_e1"], np.float32).reshape(128, 1),
        "b2": np.asarray(inputs["b_e2"], np.float32).reshape(128, 1),
        "battrf": battr_rep[:, 0:384].astype(np.float32),
        "battrb": battr_rep[:, 384:768].astype(BF),
        "Wo1": np.asarray(inputs["W_o1"], np.float32).astype(BF),
        "bo1": np.asarray(inputs["b_o1"], np.float32).reshape(128, 1),
        "Wo2": np.asarray(inputs["W_o2"]).astype(BF),
        "bo2r": np.broadcast_to(
            np.asarray(inputs["b_o2"], np.float32)[None, :], (128, 64)).copy(),
        "iota": np.broadcast_to(np.arange(128, dtype=np.float32)[None, :],
                                (128, 128)).astype(BF).copy(),
        "ones_e": np.ones((128, 1), BF),
        "ones_r": np.ones((1, 128), np.float32),
    }


def _make_in_maps(inputs, qidx, kvidx, srel):
    x = np.asarray(inputs["x"], np.float32)
    x_pad = np.zeros((NPAD, 128), BF)
    x_pad[:N] = x.astype(BF)
    in_maps = []
    for c in range(C):
        in_maps.append({
            "x_sh": x_pad[c * NSH:(c + 1) * NSH].copy(),
            "qidx": qidx[c],
            "kvidx": kvidx[c],
            "srel": srel[c],
        })
    return in_maps


def kernel(**inputs):
    from concourse import bass_utils

    TPG, NCk, qidx, kvidx, srel = _pack_edges(inputs["edge_index"])
    nc = _build_program(TPG, NCk, _make_consts(inputs))
    in_maps = _make_in_maps(inputs, qidx, kvidx, srel)
    res = bass_utils.run_bass_kernel_spmd(nc, in_maps, core_ids=list(range(C)))
    y = np.concatenate([res.results[c]["y_out"] for c in range(C)], 0)
    return y[:N].astype(np.float32)


if __name__ == "__main__":
    import pickle
    with open("/tmp/inputs.pkl", "rb") as f:
        inputs = pickle.load(f)
    y = kernel(**inputs)
    ref = np.load("/tmp/ref.npy")
    err = np.abs(y - ref).max() / np.abs(ref).max()
    print("Relative error:", err)
